# revision 4
# baseline (speedup 1.0000x reference)
"""Causal multi-head attention block (QKV proj -> causal softmax attention ->
out proj -> residual + LayerNorm) on 8 Trainium2 NeuronCores.

Sharding: core c in 0..7 -> (batch b = c//4, head-group g = c%4, heads 4g..4g+3).
Each core projects q/k/v for its 4 heads over the full 2048 rows of its batch,
runs causal attention for those heads, then an 8-way AllToAll redistributes the
attention output so core c holds rows [256c, 256c+256) of BOTH batches for the
output projection + residual + LayerNorm. Fully uniform SPMD program; all
per-core variation is carried in the input data.

Numerics: matmuls in bf16 (fp32 PSUM accumulate), softmax in fp32 on ScalarE
(unsafe softmax without max-subtraction: score sigma ~ 1/3, exp cannot
overflow), epilogue (residual + LayerNorm) in fp32. Causality is structural
(block skipping + a precomputed 0/1 triangular strip multiplied after exp), so
the dense causal_mask input never touches the device.
"""

import numpy as np
import ml_dtypes

import concourse.bass as bass
import concourse.tile as tile
from concourse import bacc, mybir
from concourse.bass_utils import run_bass_kernel_spmd

BF16 = mybir.dt.bfloat16
F32 = mybir.dt.float32
NPBF16 = ml_dtypes.bfloat16

B, S, H, NH, HD = 2, 2048, 1024, 16, 64
NCORES = 8
HPG = H // 4          # 256 hidden dims per head-group
RPC = S // NCORES     # 256 rows per core (of each batch)
KT = S // 128         # 16 key tiles of 128
QB = S // 512         # 4 query blocks of 512
EPS = 1e-5


def _build_program():
    nc = bacc.Bacc(
        "TRN2", target_bir_lowering=False, debug=False, num_devices=NCORES
    )

    # ---- I/O (per-core data, identical shapes/names on every core) ----
    xtq = nc.dram_tensor("xtq", [128, 8, S], BF16, kind="ExternalInput").ap()
    xtk = nc.dram_tensor("xtk", [128, 8, S], BF16, kind="ExternalInput").ap()
    xtv = nc.dram_tensor("xtv", [128, 8, S], BF16, kind="ExternalInput").ap()
    wqt = nc.dram_tensor("wqt", [128, 8, HPG], BF16, kind="ExternalInput").ap()
    wkt = nc.dram_tensor("wkt", [128, 8, HPG], BF16, kind="ExternalInput").ap()
    wvt = nc.dram_tensor("wvt", [128, 8, HPG], BF16, kind="ExternalInput").ap()
    bq_d = nc.dram_tensor("bq", [128, 2], F32, kind="ExternalInput").ap()
    bk_d = nc.dram_tensor("bk", [128, 2], F32, kind="ExternalInput").ap()
    bv_d = nc.dram_tensor("bv", [64, 4], F32, kind="ExternalInput").ap()
    wot = nc.dram_tensor("wot", [128, 8, H], BF16, kind="ExternalInput").ap()
    bo_d = nc.dram_tensor("bo", [H], F32, kind="ExternalInput").ap()
    lng_d = nc.dram_tensor("lng", [H], F32, kind="ExternalInput").ap()
    lnb_d = nc.dram_tensor("lnb", [H], F32, kind="ExternalInput").ap()
    qrows = nc.dram_tensor("qrows", [128, 2, 2, H], F32, kind="ExternalInput").ap()
    strip = nc.dram_tensor("strip", [128, 896], BF16, kind="ExternalInput").ap()
    out_d = nc.dram_tensor("out", [B, RPC, H], F32, kind="ExternalOutput").ap()

    with tile.TileContext(nc) as tc:
        _emit(nc, tc, locals())
    nc.compile()
    return nc


def _bcast_row(ap, parts=128):
    # [N] DRAM vector -> [parts, N] stride-0 partition broadcast AP
    return bass.AP(tensor=ap.tensor, offset=ap.offset, ap=[[0, parts]] + list(ap.ap))


def _emit(nc, tc, io):
    xtq, xtk, xtv = io["xtq"], io["xtk"], io["xtv"]
    wqt, wkt, wvt = io["wqt"], io["wkt"], io["wvt"]
    bq_d, bk_d, bv_d = io["bq_d"], io["bk_d"], io["bv_d"]
    wot, bo_d, lng_d, lnb_d = io["wot"], io["bo_d"], io["lng_d"], io["lnb_d"]
    qrows, strip, out_d = io["qrows"], io["strip"], io["out_d"]

    import contextlib

    ctx = contextlib.ExitStack()
    with ctx:
        persist = ctx.enter_context(tc.tile_pool(name="persist", bufs=1))
        xt_pool = ctx.enter_context(tc.tile_pool(name="xt", bufs=16))
        e_pool = ctx.enter_context(tc.tile_pool(name="e", bufs=4))
        a_pool = ctx.enter_context(tc.tile_pool(name="a", bufs=3))
        rc_pool = ctx.enter_context(tc.tile_pool(name="rc", bufs=2))
        rep_pool = ctx.enter_context(tc.tile_pool(name="rep", bufs=2))
        y_pool = ctx.enter_context(tc.tile_pool(name="y", bufs=2))
        st_pool = ctx.enter_context(tc.tile_pool(name="st", bufs=4))
        psum_g = ctx.enter_context(tc.tile_pool(name="psg", bufs=2, space="PSUM"))
        psum_s = ctx.enter_context(tc.tile_pool(name="pss", bufs=2, space="PSUM"))
        psum_av = ctx.enter_context(tc.tile_pool(name="psav", bufs=2, space="PSUM"))
        dram = ctx.enter_context(tc.tile_pool(name="dram", bufs=1, space="DRAM"))
        dram_sc = ctx.enter_context(tc.tile_pool(name="dram_sc", bufs=3, space="DRAM"))

        # ---- constants / persistent tiles ----
        strip_sb = persist.tile([128, 896], BF16, tag="strip")
        nc.sync.dma_start(strip_sb, strip)
        wq_sb = persist.tile([128, 8, HPG], BF16, tag="wq")
        wk_sb = persist.tile([128, 8, HPG], BF16, tag="wk")
        wv_sb = persist.tile([128, 8, HPG], BF16, tag="wv")
        nc.sync.dma_start(wq_sb, wqt)
        nc.sync.dma_start(wk_sb, wkt)
        nc.sync.dma_start(wv_sb, wvt)
        bq_sb = persist.tile([128, 2], F32, tag="bq")
        bk_sb = persist.tile([128, 2], F32, tag="bk")
        bv_sb = persist.tile([64, 4], F32, tag="bv")
        nc.sync.dma_start(bq_sb, bq_d)
        nc.sync.dma_start(bk_sb, bk_d)
        nc.sync.dma_start(bv_sb, bv_d)
        wot_sb = persist.tile([128, 8, H], BF16, tag="wot")
        nc.sync.dma_start(wot_sb, wot)
        qrows_sb = persist.tile([128, 2, 2, H], F32, tag="qrows")
        nc.sync.dma_start(qrows_sb, qrows)
        bo_rep = persist.tile([128, H], F32, tag="bo")
        g_rep = persist.tile([128, H], F32, tag="lng")
        b_rep = persist.tile([128, H], F32, tag="lnb")
        nc.gpsimd.dma_start(out=bo_rep, in_=_bcast_row(bo_d))
        nc.gpsimd.dma_start(out=g_rep, in_=_bcast_row(lng_d))
        nc.gpsimd.dma_start(out=b_rep, in_=_bcast_row(lnb_d))
        eps_sb = persist.tile([128, 1], F32, tag="eps")
        nc.vector.memset(eps_sb, EPS)

        qt_sb = persist.tile([128, 2, S], BF16, tag="qt")   # q^T  [hdim, row]
        kt_sb = persist.tile([128, 2, S], BF16, tag="kt")   # k^T  [hdim, row]
        v_sb = persist.tile([128, KT, 4, 66], BF16, tag="v")  # v natural + ones col
        nc.vector.memset(v_sb[:, :, :, 64:65], 1.0)

        # ---- phase 1: projections ----
        # q^T / k^T: out[m=hdim128, n=row] ; lhsT = W?T k-tile, rhs = x^T k-tile
        for (xt, w_sb, b_sb, o_sb) in ((xtq, wq_sb, bq_sb, qt_sb),
                                       (xtk, wk_sb, bk_sb, kt_sb)):
            xts = []
            for k in range(8):
                t = xt_pool.tile([128, S], BF16, tag="xt")
                nc.sync.dma_start(t, xt[:, k, :])
                xts.append(t)
            for m in range(2):
                pss = [psum_g.tile([128, 512], F32, tag="psg", name=f"psg_{m}_{n_}") for n_ in range(QB)]
                for k in range(8):
                    lhsT = w_sb[:, k, 128 * m:128 * m + 128]
                    for n in range(QB):
                        nc.tensor.matmul(
                            pss[n], lhsT=lhsT, rhs=xts[k][:, 512 * n:512 * n + 512],
                            start=(k == 0), stop=(k == 7),
                        )
                for n in range(QB):
                    nc.vector.tensor_scalar_add(
                        out=o_sb[:, m, 512 * n:512 * n + 512],
                        in0=pss[n], scalar1=b_sb[:, m:m + 1],
                    )
        # v natural: out[m=row128, n=vdim] ; lhsT = x^T(value) k-tile slice
        xts = []
        for k in range(8):
            t = xt_pool.tile([128, S], BF16, tag="xt")
            nc.sync.dma_start(t, xtv[:, k, :])
            xts.append(t)
        for j in range(KT):
            ps = psum_g.tile([128, 512], F32, tag="psg")
            for k in range(8):
                nc.tensor.matmul(
                    ps[:, :HPG], lhsT=xts[k][:, 128 * j:128 * j + 128],
                    rhs=wv_sb[:, k, :], start=(k == 0), stop=(k == 7),
                )
            nc.vector.tensor_copy(
                out=v_sb[:, j, :, 0:64],
                in_=ps[:, :HPG].rearrange("p (h d) -> p h d", h=4),
            )

        # ---- phase 2: causal attention, scores transposed [krow, qrow] ----
        a2a_in = dram.tile([NCORES, HPG, RPC], BF16)
        a2a_out = dram.tile([NCORES, HPG, RPC], BF16)

        for h in range(4):
            pb = 64 * (h % 2)
            chk = h // 2
            for i in range(QB):
                av = psum_av.tile([128, 512], F32, tag="psav")
                njt = 4 * i + 4  # causal: key tiles 0..4i+3
                q_rhs = qt_sb[pb:pb + 64, chk, 512 * i:512 * i + 512]
                for jp in range(njt // 2):
                    ps = psum_s.tile([128, 1024], F32, tag="pss")
                    for s in range(2):
                        j = 2 * jp + s
                        nc.tensor.matmul(
                            ps[:, 512 * s:512 * s + 512],
                            lhsT=kt_sb[pb:pb + 64, chk, 128 * j:128 * j + 128],
                            rhs=q_rhs, start=True, stop=True,
                        )
                    e = e_pool.tile([128, 1024], BF16, tag="e")
                    nc.scalar.activation(
                        out=e, in_=ps, func=mybir.ActivationFunctionType.Exp,
                        scale=float(1.0 / np.sqrt(HD)),
                    )
                    for s in range(2):
                        j = 2 * jp + s
                        u = j - 4 * i
                        if u >= 0:  # diagonal-band block: triangular mask
                            off = 384 - 128 * u
                            nc.vector.tensor_mul(
                                out=e[:, 512 * s:512 * s + 512],
                                in0=e[:, 512 * s:512 * s + 512],
                                in1=strip_sb[:, off:off + 512],
                            )
                        nc.tensor.matmul(
                            av[:65, :], lhsT=v_sb[:, j, h, 0:65],
                            rhs=e[:, 512 * s:512 * s + 512],
                            start=(j == 0), stop=(j == njt - 1),
                        )
                # normalize: row 64 of av is the softmax denominator per qrow
                rc = rc_pool.tile([128, 512], F32, tag="rc")
                nc.vector.reciprocal(rc[64:65, :], av[64:65, :])
                # partition-broadcast via DRAM bounce (SBUF APs can't have
                # stride-0 partition dims; DRAM APs can)
                dnm = dram_sc.tile([1, 512], F32, name=f"dnm_{h}_{i}")
                nc.sync.dma_start(dnm, rc[64:65, :])
                rep = rep_pool.tile([64, 512], F32, tag="rep")
                nc.sync.dma_start(
                    rep,
                    bass.AP(tensor=dnm.tensor, offset=dnm.offset,
                            ap=[[0, 64]] + list(dnm.ap[1:])),
                )
                at = a_pool.tile([64, 512], BF16, tag="at")
                nc.vector.tensor_mul(out=at, in0=av[:64, :], in1=rep)
                nc.vector.tensor_scalar_add(out=at, in0=at, scalar1=bv_sb[:, h:h + 1])
                for s in range(2):
                    nc.sync.dma_start(
                        a2a_in[2 * i + s, 64 * h:64 * h + 64, :],
                        at[:, 256 * s:256 * s + 256],
                    )

        # ---- phase 3: all-to-all + out projection + residual + LN ----
        nc.gpsimd.collective_compute(
            "AllToAll", mybir.AluOpType.bypass,
            replica_groups=[list(range(NCORES))],
            ins=[a2a_in[:].opt()], outs=[a2a_out[:].opt()],
        )
        lh = persist.tile([128, 2, 8, RPC], BF16, tag="lh")
        for b in range(2):
            for gg in range(4):
                for s2 in range(2):
                    nc.sync.dma_start(
                        lh[:, b, 2 * gg + s2, :],
                        a2a_out[4 * b + gg, 128 * s2:128 * s2 + 128, :],
                    )
        pre = persist.tile([128, 2, 2, H], F32, tag="pre")
        for b in range(2):
            for m in range(2):
                nc.vector.tensor_add(
                    out=pre[:, b, m, :], in0=qrows_sb[:, b, m, :], in1=bo_rep
                )
        for b in range(2):
            for m in range(2):
                pso = [psum_g.tile([128, 512], F32, tag="psg", name=f"pso_{b}_{m}_{n_}") for n_ in range(2)]
                for k in range(8):
                    lhsT = lh[:, b, k, 128 * m:128 * m + 128]
                    for n in range(2):
                        nc.tensor.matmul(
                            pso[n], lhsT=lhsT, rhs=wot_sb[:, k, 512 * n:512 * n + 512],
                            start=(k == 0), stop=(k == 7),
                        )
                y = y_pool.tile([128, H], F32, tag="y")
                for n in range(2):
                    nc.vector.tensor_add(
                        out=y[:, 512 * n:512 * n + 512], in0=pso[n],
                        in1=pre[:, b, m, 512 * n:512 * n + 512],
                    )
                st = st_pool.tile([128, 2, 6], F32, tag="st")
                for cch in range(2):
                    nc.vector.bn_stats(st[:, cch, :], y[:, 512 * cch:512 * cch + 512])
                mv = st_pool.tile([128, 2], F32, tag="mv")
                nc.vector.bn_aggr(mv, st)
                rstd = st_pool.tile([128, 1], F32, tag="rstd")
                nc.scalar.activation(
                    out=rstd, in_=mv[:, 1:2],
                    func=mybir.ActivationFunctionType.Sqrt, bias=eps_sb,
                )
                nc.vector.reciprocal(rstd, rstd)
                nc.vector.tensor_scalar(
                    out=y, in0=y, scalar1=mv[:, 0:1], scalar2=rstd,
                    op0=mybir.AluOpType.subtract, op1=mybir.AluOpType.mult,
                )
                nc.vector.tensor_mul(out=y, in0=y, in1=g_rep)
                nc.vector.tensor_add(out=y, in0=y, in1=b_rep)
                nc.sync.dma_start(out_d[b, 128 * m:128 * m + 128, :], y)


_CACHE = {}


def _get_program():
    if "nc" not in _CACHE:
        _CACHE["nc"] = _build_program()
    return _CACHE["nc"]


def _prep_in_maps(query, key, value, Wq, bq, Wk, bk, Wv, bv, Wo, bo, ln_g, ln_b):
    def part(a):  # [128k, N] -> [128, k, N] partition-inner layout
        d0, d1 = a.shape
        return np.ascontiguousarray(
            a.reshape(d0 // 128, 128, d1).transpose(1, 0, 2)
        )

    def t_bf16(a):
        return part(np.ascontiguousarray(a.T).astype(NPBF16))

    WqT, WkT, WvT = Wq.T, Wk.T, Wv.T  # [H_in, H_out]
    xts = [[t_bf16(x[b]) for b in range(B)] for x in (query, key, value)]
    wotp = part(np.ascontiguousarray(Wo.T).astype(NPBF16))
    strip = (
        np.arange(896, dtype=np.int64)[None, :]
        >= (np.arange(128, dtype=np.int64)[:, None] + 384)
    ).astype(NPBF16)

    in_maps = []
    for c in range(NCORES):
        b, g = c // 4, c % 4
        sl = slice(HPG * g, HPG * (g + 1))
        qr = query[:, RPC * c:RPC * (c + 1), :].astype(np.float32)
        qr = np.ascontiguousarray(
            qr.reshape(B, 2, 128, H).transpose(2, 0, 1, 3)
        )
        in_maps.append({
            "xtq": xts[0][b], "xtk": xts[1][b], "xtv": xts[2][b],
            "wqt": part(np.ascontiguousarray(WqT[:, sl]).astype(NPBF16)),
            "wkt": part(np.ascontiguousarray(WkT[:, sl]).astype(NPBF16)),
            "wvt": part(np.ascontiguousarray(WvT[:, sl]).astype(NPBF16)),
            "bq": np.ascontiguousarray(
                bq[sl].astype(np.float32).reshape(2, 128).T),
            "bk": np.ascontiguousarray(
                bk[sl].astype(np.float32).reshape(2, 128).T),
            "bv": np.ascontiguousarray(
                bv[sl].astype(np.float32).reshape(4, 64).T),
            "wot": wotp,
            "bo": bo.astype(np.float32), "lng": ln_g.astype(np.float32),
            "lnb": ln_b.astype(np.float32),
            "qrows": qr, "strip": strip,
        })
    return in_maps


def kernel(query, key, value, causal_mask, Wq, bq, Wk, bk, Wv, bv, Wo, bo,
           ln_g, ln_b, _trace=False, _trace_cores=None):
    query, key, value = (np.asarray(x, np.float32) for x in (query, key, value))
    Wq, bq, Wk, bk, Wv, bv, Wo, bo, ln_g, ln_b = (
        np.asarray(x, np.float32)
        for x in (Wq, bq, Wk, bk, Wv, bv, Wo, bo, ln_g, ln_b)
    )
    nc = _get_program()
    in_maps = _prep_in_maps(
        query, key, value, Wq, bq, Wk, bk, Wv, bv, Wo, bo, ln_g, ln_b
    )
    kwargs = {}
    if _trace:
        kwargs = dict(trace=True)
        if _trace_cores is not None:
            kwargs["trace_cores"] = _trace_cores
    res = run_bass_kernel_spmd(nc, in_maps, core_ids=list(range(NCORES)), **kwargs)
    _CACHE["last_results"] = res
    out = np.empty((B, S, H), np.float32)
    for c in range(NCORES):
        out[:, RPC * c:RPC * (c + 1), :] = res.results[c]["out"]
    return out


# revision 35
# speedup vs baseline: 478.0418x; 478.0418x over previous
"""Causal multi-head attention block (QKV proj -> causal softmax attention ->
out proj -> residual + LayerNorm) on 8 Trainium2 NeuronCores.

Sharding: core c in 0..7 -> (batch b = c//4, head-group g = c%4, heads 4g..4g+3).
Each core projects q/k/v for its 4 heads over the full 2048 rows of its batch,
runs causal attention for those heads, then an 8-way AllToAll redistributes the
attention output so core c holds rows [256c, 256c+256) of BOTH batches for the
output projection + residual + LayerNorm. Fully uniform SPMD program; all
per-core variation is carried in the input data.

Numerics: matmuls in bf16 (fp32 PSUM accumulate), softmax in fp32 on ScalarE
(unsafe softmax without max-subtraction: score sigma ~ 1/3, exp cannot
overflow), epilogue (residual + LayerNorm) in fp32. Causality is structural
(block skipping + a precomputed 0/1 triangular strip multiplied after exp), so
the dense causal_mask input never touches the device.
"""

import numpy as np
import ml_dtypes

import concourse.bass as bass
import concourse.tile as tile
from concourse import bacc, mybir
from concourse.bass_utils import run_bass_kernel_spmd

BF16 = mybir.dt.bfloat16
F32 = mybir.dt.float32
NPBF16 = ml_dtypes.bfloat16

B, S, H, NH, HD = 2, 2048, 1024, 16, 64
NCORES = 8
HPG = H // 4          # 256 hidden dims per head-group
RPC = S // NCORES     # 256 rows per core (of each batch)
KT = S // 128         # 16 key tiles of 128
QB = S // 512         # 4 query blocks of 512
EPS = 1e-5


def _build_program(sim=False, reps=1):
    nc = bacc.Bacc(
        "TRN2", target_bir_lowering=False, debug=False, num_devices=NCORES
    )

    # ---- I/O (per-core data, identical shapes/names on every core) ----
    xtq = nc.dram_tensor("xtq", [128, 8, S], BF16, kind="ExternalInput").ap()
    xtk = nc.dram_tensor("xtk", [128, 8, S], BF16, kind="ExternalInput").ap()
    xtv = nc.dram_tensor("xtv", [128, 8, S], BF16, kind="ExternalInput").ap()
    wqt = nc.dram_tensor("wqt", [128, 8, HPG], BF16, kind="ExternalInput").ap()
    wkt = nc.dram_tensor("wkt", [128, 8, HPG], BF16, kind="ExternalInput").ap()
    wvt = nc.dram_tensor("wvt", [128, 8, HPG], BF16, kind="ExternalInput").ap()
    bq_d = nc.dram_tensor("bq", [128, 2], F32, kind="ExternalInput").ap()
    bk_d = nc.dram_tensor("bk", [128, 2], F32, kind="ExternalInput").ap()
    bv_d = nc.dram_tensor("bv", [64, 4], F32, kind="ExternalInput").ap()
    wot = nc.dram_tensor("wot", [128, 8, H], BF16, kind="ExternalInput").ap()
    bo_d = nc.dram_tensor("bo", [H], F32, kind="ExternalInput").ap()
    lng_d = nc.dram_tensor("lng", [H], F32, kind="ExternalInput").ap()
    lnb_d = nc.dram_tensor("lnb", [H], F32, kind="ExternalInput").ap()
    qrows = nc.dram_tensor("qrows", [128, 2, 2, H], F32, kind="ExternalInput").ap()
    # packed diagonal-band mask: concat over u=0..3 of [x >= p, x < 512-128u]
    strip = nc.dram_tensor("strip", [128, 1280], BF16, kind="ExternalInput").ap()
    out_d = nc.dram_tensor("out", [B, RPC, H], F32, kind="ExternalOutput").ap()

    io = locals()
    with tile.TileContext(nc) as tc:
        for _rep in range(reps):
            _emit(nc, tc, io, sim=sim)
    nc.compile()
    return nc


def _bcast_row(ap, parts=128):
    # [N] DRAM vector -> [parts, N] stride-0 partition broadcast AP
    return bass.AP(tensor=ap.tensor, offset=ap.offset, ap=[[0, parts]] + list(ap.ap))


def _emit(nc, tc, io, sim=False):
    xtq, xtk, xtv = io["xtq"], io["xtk"], io["xtv"]
    wqt, wkt, wvt = io["wqt"], io["wkt"], io["wvt"]
    bq_d, bk_d, bv_d = io["bq_d"], io["bk_d"], io["bv_d"]
    wot, bo_d, lng_d, lnb_d = io["wot"], io["bo_d"], io["lng_d"], io["lnb_d"]
    qrows, strip, out_d = io["qrows"], io["strip"], io["out_d"]

    import contextlib

    ctx = contextlib.ExitStack()
    with ctx:
        persist = ctx.enter_context(tc.tile_pool(name="persist", bufs=1))
        xt_pool = ctx.enter_context(tc.tile_pool(name="xt", bufs=16))
        e_pool = ctx.enter_context(tc.tile_pool(name="e", bufs=4))
        a_pool = ctx.enter_context(tc.tile_pool(name="a", bufs=3))
        rc_pool = ctx.enter_context(tc.tile_pool(name="rc", bufs=2))
        rep_pool = ctx.enter_context(tc.tile_pool(name="rep", bufs=2))
        y_pool = ctx.enter_context(tc.tile_pool(name="y", bufs=4))
        st_pool = ctx.enter_context(tc.tile_pool(name="st", bufs=4))
        psum_g = ctx.enter_context(tc.tile_pool(name="psg", bufs=2, space="PSUM"))
        psum_s = ctx.enter_context(tc.tile_pool(name="pss", bufs=2, space="PSUM"))
        psum_av = ctx.enter_context(tc.tile_pool(name="psav", bufs=2, space="PSUM"))
        dram = ctx.enter_context(tc.tile_pool(name="dram", bufs=1, space="DRAM"))
        dram_sc = ctx.enter_context(tc.tile_pool(name="dram_sc", bufs=3, space="DRAM"))

        # ---- constants needed for phase 1 (small, load first) ----
        wq_sb = persist.tile([128, 8, HPG], BF16, tag="wq")
        wk_sb = persist.tile([128, 8, HPG], BF16, tag="wk")
        wv_sb = persist.tile([128, 8, HPG], BF16, tag="wv")
        nc.sync.dma_start(wv_sb, wvt)
        nc.sync.dma_start(wq_sb, wqt)
        nc.sync.dma_start(wk_sb, wkt)
        strip_sb = persist.tile([128, 1280], BF16, tag="strip")
        nc.sync.dma_start(strip_sb, strip)
        bq_sb = persist.tile([128, 2], F32, tag="bq")
        bk_sb = persist.tile([128, 2], F32, tag="bk")
        bv_sb = persist.tile([64, 4], F32, tag="bv")
        nc.sync.dma_start(bq_sb, bq_d)
        nc.sync.dma_start(bk_sb, bk_d)
        nc.sync.dma_start(bv_sb, bv_d)
        eps_sb = persist.tile([128, 1], F32, tag="eps")
        nc.vector.memset(eps_sb, EPS)

        qt_sb = persist.tile([128, 2, S], BF16, tag="qt")   # q^T  [hdim, row]
        kt_sb = persist.tile([128, 2, S], BF16, tag="kt")   # k^T  [hdim, row]
        v_sb = persist.tile([128, KT, 4, 66], BF16, tag="v")  # v natural + ones col
        nc.vector.memset(v_sb[:, :, :, 64:65], 1.0)

        # split exchange: heads 0-1 (lo) fly while heads 2-3 (hi) compute
        a2a_in_lo = dram.tile([NCORES, 128, RPC], BF16)
        a2a_out_lo = dram.tile([NCORES, 128, RPC], BF16)
        a2a_in_hi = dram.tile([NCORES, 128, RPC], BF16)
        a2a_out_hi = dram.tile([NCORES, 128, RPC], BF16)
        ESC = float(1.0 / np.sqrt(HD))

        def attention(h, i):
            """Causal attention for one head h (0-3) and one 512-row q block."""
            pb = 64 * (h % 2)
            chk = h // 2
            av = psum_av.tile([128, 512], F32, tag="psav", name=f"av_{h}_{i}")
            q_rhs = qt_sb[pb:pb + 64, chk, 512 * i:512 * i + 512]

            def kslice(j):
                return kt_sb[pb:pb + 64, chk, 128 * j:128 * j + 128]

            # fully-unmasked key tiles (j < 4i), two per fp32 PSUM tile so
            # one wide exp amortizes the ACT fixed cost
            for jp in range(2 * i):
                ps = psum_s.tile([128, 1024], F32, tag="pss",
                                 name=f"pss_{h}_{i}_{jp}")
                for s in range(2):
                    nc.tensor.matmul(
                        ps[:, 512 * s:512 * s + 512], lhsT=kslice(2 * jp + s),
                        rhs=q_rhs, start=True, stop=True,
                    )
                e = e_pool.tile([128, 1024], BF16, tag="e",
                                name=f"e_{h}_{i}_{jp}")
                nc.scalar.activation(
                    out=e, in_=ps, func=mybir.ActivationFunctionType.Exp,
                    scale=ESC,
                )
                for s in range(2):
                    j = 2 * jp + s
                    nc.tensor.matmul(
                        av[:65, :], lhsT=v_sb[:, j, h, 0:65],
                        rhs=e[:, 512 * s:512 * s + 512],
                        start=(j == 0), stop=False,
                    )
            # diagonal band (j = 4i..4i+3, u = j-4i): columns qcol < 128u
            # are fully masked; compute/exp only the live ranges, packed:
            # tile A: u0@0 (512) + u1@512 (384); tile B: u2@0 (256) + u3@256 (128)
            for dp in range(2):
                u0 = 2 * dp
                w0, w1 = 512 - 128 * u0, 384 - 128 * u0
                wt = w0 + w1
                soff = 896 * dp  # packed-strip offset for this tile
                ps = psum_s.tile([128, 1024], F32, tag="pss",
                                 name=f"pssd_{h}_{i}_{dp}")
                nc.tensor.matmul(
                    ps[:, 0:w0], lhsT=kslice(4 * i + u0),
                    rhs=q_rhs[:, 128 * u0:512], start=True, stop=True,
                )
                nc.tensor.matmul(
                    ps[:, w0:wt], lhsT=kslice(4 * i + u0 + 1),
                    rhs=q_rhs[:, 128 * u0 + 128:512], start=True, stop=True,
                )
                e = e_pool.tile([128, 1024], BF16, tag="e",
                                name=f"ed_{h}_{i}_{dp}")
                nc.scalar.activation(
                    out=e[:, 0:wt], in_=ps[:, 0:wt],
                    func=mybir.ActivationFunctionType.Exp, scale=ESC,
                )
                nc.vector.tensor_mul(
                    out=e[:, 0:wt], in0=e[:, 0:wt],
                    in1=strip_sb[:, soff:soff + wt],
                )
                nc.tensor.matmul(
                    av[:65, 128 * u0:512], lhsT=v_sb[:, 4 * i + u0, h, 0:65],
                    rhs=e[:, 0:w0],
                    start=(4 * i + u0 == 0), stop=False,
                )
                nc.tensor.matmul(
                    av[:65, 128 * u0 + 128:512],
                    lhsT=v_sb[:, 4 * i + u0 + 1, h, 0:65],
                    rhs=e[:, w0:wt],
                    start=False, stop=(dp == 1),
                )
            # normalize: row 64 of av is the softmax denominator per qrow.
            # Evacuate the PSUM tile immediately (two quick DVE reads) so
            # the av slot frees before the DRAM-bounce broadcast.
            atu = a_pool.tile([64, 512], BF16, tag="atu", name=f"atu_{h}_{i}")
            nc.vector.tensor_copy(out=atu, in_=av[:64, :])
            rc = rc_pool.tile([128, 512], F32, tag="rc", name=f"rc_{h}_{i}")
            nc.vector.reciprocal(rc[64:65, :], av[64:65, :])
            # partition-broadcast via DRAM bounce (SBUF APs can't have
            # stride-0 partition dims; DRAM APs can)
            dnm = dram_sc.tile([1, 512], F32, name=f"dnm_{h}_{i}")
            nc.sync.dma_start(dnm, rc[64:65, :])
            rep = rep_pool.tile([64, 512], F32, tag="rep", name=f"rep_{h}_{i}")
            nc.sync.dma_start(
                rep,
                bass.AP(tensor=dnm.tensor, offset=dnm.offset,
                        ap=[[0, 64]] + list(dnm.ap[1:])),
            )
            at = a_pool.tile([64, 512], BF16, tag="at", name=f"at_{h}_{i}")
            nc.vector.tensor_mul(out=at, in0=atu, in1=rep)
            nc.vector.tensor_scalar_add(out=at, in0=at, scalar1=bv_sb[:, h:h + 1])
            a2a_in = a2a_in_lo if h < 2 else a2a_in_hi
            for s in range(2):
                nc.sync.dma_start(
                    a2a_in[2 * i + s, 64 * (h % 2):64 * (h % 2) + 64, :],
                    at[:, 256 * s:256 * s + 256],
                )

        def exchange(half):
            ain = a2a_in_lo if half == 0 else a2a_in_hi
            aout = a2a_out_lo if half == 0 else a2a_out_hi
            if sim:
                # single-core sim: stand in for the collective with a
                # same-size DRAM->DRAM copy (wrong data, same deps)
                nc.sync.dma_start(aout[:], ain[:])
            else:
                nc.gpsimd.collective_compute(
                    "AllToAll", mybir.AluOpType.bypass,
                    replica_groups=[list(range(NCORES))],
                    ins=[ain[:].opt()], outs=[aout[:].opt()],
                )

        # ---- phase 1 + 2 interleaved ----
        # v first (its lhsT tiles stream in earliest, PE starts ~immediately),
        # then k^T fully, then q^T chunk-by-chunk with heads 0-1 attention
        # interleaved so the ScalarE exp pipeline starts ~35us earlier.
        xts = []
        for k in range(8):
            t = xt_pool.tile([128, S], BF16, tag="xt", name=f"xtv_{k}")
            nc.sync.dma_start(t, xtv[:, k, :])
            xts.append(t)
        for j in range(KT):
            ps = psum_g.tile([128, 512], F32, tag="psg")
            for k in range(8):
                nc.tensor.matmul(
                    ps[:, :HPG], lhsT=xts[k][:, 128 * j:128 * j + 128],
                    rhs=wv_sb[:, k, :], start=(k == 0), stop=(k == 7),
                )
            nc.vector.tensor_copy(
                out=v_sb[:, j, :, 0:64],
                in_=ps[:, :HPG].rearrange("p (h d) -> p h d", h=4),
            )
        # k^T fully
        xtks = []
        for k in range(8):
            t = xt_pool.tile([128, S], BF16, tag="xt", name=f"xtk_{k}")
            nc.sync.dma_start(t, xtk[:, k, :])
            xtks.append(t)
        for n in range(QB):
            for m in range(2):
                ps = psum_g.tile([128, 512], F32, tag="psg",
                                 name=f"psg_k_{n}_{m}")
                for k in range(8):
                    nc.tensor.matmul(
                        ps, lhsT=wk_sb[:, k, 128 * m:128 * m + 128],
                        rhs=xtks[k][:, 512 * n:512 * n + 512],
                        start=(k == 0), stop=(k == 7),
                    )
                nc.vector.tensor_scalar_add(
                    out=kt_sb[:, m, 512 * n:512 * n + 512],
                    in0=ps, scalar1=bk_sb[:, m:m + 1],
                )
        # q^T chunk n=i feeds attention(h=0/1, i) immediately
        xtqs = []
        for k in range(8):
            t = xt_pool.tile([128, S], BF16, tag="xt", name=f"xtq_{k}")
            nc.sync.dma_start(t, xtq[:, k, :])
            xtqs.append(t)
        for n in range(QB):
            for m in range(2):
                ps = psum_g.tile([128, 512], F32, tag="psg",
                                 name=f"psg_q_{n}_{m}")
                for k in range(8):
                    nc.tensor.matmul(
                        ps, lhsT=wq_sb[:, k, 128 * m:128 * m + 128],
                        rhs=xtqs[k][:, 512 * n:512 * n + 512],
                        start=(k == 0), stop=(k == 7),
                    )
                nc.vector.tensor_scalar_add(
                    out=qt_sb[:, m, 512 * n:512 * n + 512],
                    in0=ps, scalar1=bq_sb[:, m:m + 1],
                )
            attention(0, n)
            attention(1, n)
        exchange(0)

        # phase-3 constants: emitted here so their DMAs fill the bus while
        # heads 2-3 attention runs (DMA is nearly idle during phase 2)
        wot_sb = persist.tile([128, 8, H], BF16, tag="wot")
        nc.sync.dma_start(wot_sb, wot)
        qrows_sb = persist.tile([128, 2, 2, H], F32, tag="qrows")
        nc.sync.dma_start(qrows_sb, qrows)
        bo_rep = persist.tile([128, H], F32, tag="bo")
        g_rep = persist.tile([128, H], F32, tag="lng")
        b_rep = persist.tile([128, H], F32, tag="lnb")
        nc.gpsimd.dma_start(out=bo_rep, in_=_bcast_row(bo_d))
        nc.gpsimd.dma_start(out=g_rep, in_=_bcast_row(lng_d))
        nc.gpsimd.dma_start(out=b_rep, in_=_bcast_row(lnb_d))
        # pre = qrows + bo, computed in place (qrows has no other reader)
        pre = qrows_sb
        for b in range(2):
            for m in range(2):
                nc.vector.tensor_add(
                    out=pre[:, b, m, :], in0=qrows_sb[:, b, m, :], in1=bo_rep
                )

        lh = persist.tile([128, 2, 8, RPC], BF16, tag="lh")
        for b in range(2):
            for gg in range(4):
                nc.sync.dma_start(
                    lh[:, b, 2 * gg, :], a2a_out_lo[4 * b + gg, :, :]
                )
        for h in (2, 3):
            for i in range(QB):
                attention(h, i)

        # lo-half out projection (ksub even): emitted after attention so its
        # matmuls backfill the PE idle while the last heads' softmax drains;
        # partials parked in SBUF y tiles
        ys = {}
        for b in range(2):
            for m in range(2):
                pso = [psum_g.tile([128, 512], F32, tag="psg",
                                   name=f"psoA_{b}_{m}_{n_}") for n_ in range(2)]
                for k in range(0, 8, 2):
                    lhsT = lh[:, b, k, 128 * m:128 * m + 128]
                    for n in range(2):
                        nc.tensor.matmul(
                            pso[n], lhsT=lhsT,
                            rhs=wot_sb[:, k, 512 * n:512 * n + 512],
                            start=(k == 0), stop=(k == 6),
                        )
                y = y_pool.tile([128, H], F32, tag="y", name=f"y_{b}_{m}")
                ys[(b, m)] = y
                for n in range(2):
                    nc.vector.tensor_add(
                        out=y[:, 512 * n:512 * n + 512], in0=pso[n],
                        in1=pre[:, b, m, 512 * n:512 * n + 512],
                    )
        exchange(1)

        # ---- phase 3: hi-half out projection + residual + LN ----
        for b in range(2):
            for gg in range(4):
                nc.sync.dma_start(
                    lh[:, b, 2 * gg + 1, :], a2a_out_hi[4 * b + gg, :, :]
                )
        for b in range(2):
            for m in range(2):
                pso = [psum_g.tile([128, 512], F32, tag="psg",
                                   name=f"psoB_{b}_{m}_{n_}") for n_ in range(2)]
                for k in range(1, 8, 2):
                    lhsT = lh[:, b, k, 128 * m:128 * m + 128]
                    for n in range(2):
                        nc.tensor.matmul(
                            pso[n], lhsT=lhsT,
                            rhs=wot_sb[:, k, 512 * n:512 * n + 512],
                            start=(k == 1), stop=(k == 7),
                        )
                y = ys[(b, m)]
                for n in range(2):
                    nc.vector.tensor_add(
                        out=y[:, 512 * n:512 * n + 512],
                        in0=y[:, 512 * n:512 * n + 512], in1=pso[n],
                    )
                st = st_pool.tile([128, 2, 6], F32, tag="st")
                for cch in range(2):
                    nc.vector.bn_stats(st[:, cch, :], y[:, 512 * cch:512 * cch + 512])
                mv = st_pool.tile([128, 2], F32, tag="mv")
                nc.vector.bn_aggr(mv, st)
                rstd = st_pool.tile([128, 1], F32, tag="rstd")
                nc.scalar.activation(
                    out=rstd, in_=mv[:, 1:2],
                    func=mybir.ActivationFunctionType.Sqrt, bias=eps_sb,
                )
                nc.vector.reciprocal(rstd, rstd)
                # nmr = -mean * rstd, then ACT applies (y*rstd + nmr) in one
                # Identity pass (per-partition scale/bias APs) - keeps the
                # wide op off the busy VectorE
                nmr = st_pool.tile([128, 1], F32, tag="nmr")
                nc.vector.tensor_scalar(
                    out=nmr, in0=mv[:, 0:1], scalar1=rstd, scalar2=-1.0,
                    op0=mybir.AluOpType.mult, op1=mybir.AluOpType.mult,
                )
                nc.scalar.activation(
                    out=y, in_=y, func=mybir.ActivationFunctionType.Identity,
                    bias=nmr, scale=rstd,
                )
                nc.vector.tensor_mul(out=y, in0=y, in1=g_rep)
                nc.vector.tensor_add(out=y, in0=y, in1=b_rep)
                nc.sync.dma_start(out_d[b, 128 * m:128 * m + 128, :], y)


_CACHE = {}


def _get_program():
    if "nc" not in _CACHE:
        _CACHE["nc"] = _build_program()
    return _CACHE["nc"]


def _prep_in_maps(query, key, value, Wq, bq, Wk, bk, Wv, bv, Wo, bo, ln_g, ln_b):
    def part(a):  # [128k, N] -> [128, k, N] partition-inner layout
        d0, d1 = a.shape
        return np.ascontiguousarray(
            a.reshape(d0 // 128, 128, d1).transpose(1, 0, 2)
        )

    def t_bf16(a):
        return part(np.ascontiguousarray(a.T).astype(NPBF16))

    WqT, WkT, WvT = Wq.T, Wk.T, Wv.T  # [H_in, H_out]
    xts = [[t_bf16(x[b]) for b in range(B)] for x in (query, key, value)]
    wotp = part(np.ascontiguousarray(Wo.T).astype(NPBF16))
    # packed diagonal masks: for u in 0..3, width 512-128u, keep x >= p
    p_idx = np.arange(128, dtype=np.int64)[:, None]
    strip = np.concatenate(
        [(np.arange(512 - 128 * u, dtype=np.int64)[None, :] >= p_idx)
         for u in range(4)], axis=1,
    ).astype(NPBF16)
    assert strip.shape == (128, 1280)

    in_maps = []
    for c in range(NCORES):
        b, g = c // 4, c % 4
        sl = slice(HPG * g, HPG * (g + 1))
        qr = query[:, RPC * c:RPC * (c + 1), :].astype(np.float32)
        qr = np.ascontiguousarray(
            qr.reshape(B, 2, 128, H).transpose(2, 0, 1, 3)
        )
        in_maps.append({
            "xtq": xts[0][b], "xtk": xts[1][b], "xtv": xts[2][b],
            "wqt": part(np.ascontiguousarray(WqT[:, sl]).astype(NPBF16)),
            "wkt": part(np.ascontiguousarray(WkT[:, sl]).astype(NPBF16)),
            "wvt": part(np.ascontiguousarray(WvT[:, sl]).astype(NPBF16)),
            "bq": np.ascontiguousarray(
                bq[sl].astype(np.float32).reshape(2, 128).T),
            "bk": np.ascontiguousarray(
                bk[sl].astype(np.float32).reshape(2, 128).T),
            "bv": np.ascontiguousarray(
                bv[sl].astype(np.float32).reshape(4, 64).T),
            "wot": wotp,
            "bo": bo.astype(np.float32), "lng": ln_g.astype(np.float32),
            "lnb": ln_b.astype(np.float32),
            "qrows": qr, "strip": strip,
        })
    return in_maps


def kernel(query, key, value, causal_mask, Wq, bq, Wk, bk, Wv, bv, Wo, bo,
           ln_g, ln_b, _trace=False, _trace_cores=None):
    query, key, value = (np.asarray(x, np.float32) for x in (query, key, value))
    Wq, bq, Wk, bk, Wv, bv, Wo, bo, ln_g, ln_b = (
        np.asarray(x, np.float32)
        for x in (Wq, bq, Wk, bk, Wv, bv, Wo, bo, ln_g, ln_b)
    )
    nc = _get_program()
    in_maps = _prep_in_maps(
        query, key, value, Wq, bq, Wk, bk, Wv, bv, Wo, bo, ln_g, ln_b
    )
    kwargs = {}
    if _trace:
        kwargs = dict(trace=True)
        if _trace_cores is not None:
            kwargs["trace_cores"] = _trace_cores
    res = run_bass_kernel_spmd(nc, in_maps, core_ids=list(range(NCORES)), **kwargs)
    _CACHE["last_results"] = res
    out = np.empty((B, S, H), np.float32)
    for c in range(NCORES):
        out[:, RPC * c:RPC * (c + 1), :] = res.results[c]["out"]
    return out
